# revision 1
# baseline (speedup 1.0000x reference)
"""AdditiveAttention kernel for one TRN2 chip (8 NeuronCores).

Reference computation (per batch b):
    q = queries @ W_q                         # (NQ, H)
    k = keys @ W_k                            # (NK, H)
    scores[i,j] = sum_h v_w[h] * tanh(q[i,h] + k[j,h])
    out = masked_softmax(scores, valid_len) @ values

Sharding: data-parallel over (batch, query-half): core c handles batch c//2,
query rows (c%2)*64 .. +64.  All compute is core-local (no collectives);
the host does layout prep (transposes / masking / padding) and reassembly.

Device dataflow per core (64 queries x 1024 keys x H=256):
  P1  kh[h,j] = W_k.T @ keys.T, qh[h,i] = W_q.T @ queries.T     (PE, bf16)
  P2  for each query i: feat = tanh(kh + qh[:,i]) via ScalarE bias-add;
      scoresT[j,i] += feat[h,jblk].T @ v_w  (PE, feat as stationary operand)
      -> scores accumulate TRANSPOSED in PSUM, [j,i] layout
  P3  wT = exp(scoresT)  (no max subtraction needed: |score| <= sum|v_w| ~ 13)
  P4  out_aug[i,:] = sum_j wT[j,i] * va[j,:]  where va = [masked values | mask]
      -> column 256 of out_aug is the softmax denominator
  P5  out = out_aug[:, :256] * (1 / out_aug[:, 256])
"""

import numpy as np
import ml_dtypes

import concourse.bass as bass
import concourse.tile as tile
from concourse import bacc, mybir
from concourse.bass_utils import run_bass_kernel_spmd

BF16 = mybir.dt.bfloat16
F32 = mybir.dt.float32
NP_BF16 = ml_dtypes.bfloat16

B, NQ, NK, DQ, DK, H, DV = 4, 128, 1024, 256, 256, 256, 256
NQC = NQ // 2  # queries per core
VA_W = 258  # values (256) + mask column (1) + pad (1)
N_CORES = 8

_CACHED_NC = None


def build_kernel():
    """Build + compile the per-core Bass graph (SPMD across 8 cores)."""
    nc = bacc.Bacc("TRN2", target_bir_lowering=False, debug=False, num_devices=N_CORES)

    qT_d = nc.declare_dram_parameter("qT", [2, 128, NQC], BF16, isOutput=False)
    kT_d = nc.declare_dram_parameter("kT", [2, 128, NK], BF16, isOutput=False)
    wq_d = nc.declare_dram_parameter("wq", [2, 128, H], BF16, isOutput=False)
    wk_d = nc.declare_dram_parameter("wk", [2, 128, H], BF16, isOutput=False)
    vw_d = nc.declare_dram_parameter("vw", [128, 2], BF16, isOutput=False)
    va_d = nc.declare_dram_parameter("va", [8, 128, VA_W], BF16, isOutput=False)
    out_d = nc.declare_dram_parameter("out", [NQC, DV], F32, isOutput=True)

    Tanh = mybir.ActivationFunctionType.Tanh
    Exp = mybir.ActivationFunctionType.Exp

    with tile.TileContext(nc) as tc:
        with tc.tile_pool(name="const", bufs=1) as cpool:
            wk_sb = cpool.tile([128, 2, H], BF16)
            wq_sb = cpool.tile([128, 2, H], BF16)
            kT_sb = cpool.tile([128, 2, NK], BF16)
            qT_sb = cpool.tile([128, 2, NQC], BF16)
            vw_sb = cpool.tile([128, 2], BF16)
            va_sb = cpool.tile([128, 8, VA_W], BF16)
            for t in range(2):
                nc.sync.dma_start(out=wk_sb[:, t, :], in_=wk_d[t])
                nc.sync.dma_start(out=wq_sb[:, t, :], in_=wq_d[t])
                nc.sync.dma_start(out=kT_sb[:, t, :], in_=kT_d[t])
                nc.sync.dma_start(out=qT_sb[:, t, :], in_=qT_d[t])
            nc.sync.dma_start(out=vw_sb, in_=vw_d[:, :])
            for t in range(8):
                nc.sync.dma_start(out=va_sb[:, t, :], in_=va_d[t])

            kh_sb = cpool.tile([128, 2, NK], F32)
            qh_sb = cpool.tile([128, 2 * NQC], F32)
            wT_sb = cpool.tile([128, 8, NQC], BF16)
            out_sb = cpool.tile([NQC, DV], F32)
            rsum = cpool.tile([NQC, 1], F32)

            # ---- P1: projections: kh[h, j], qh[h, i] (h on partitions) ----
            with tc.tile_pool(name="proj_psum", bufs=2, space="PSUM") as pp:
                for ht in range(2):
                    ps_k = pp.tile([128, NK], F32, tag="ps_k")
                    for jh in range(2):
                        for dt in range(2):
                            nc.tensor.matmul(
                                ps_k[:, jh * 512 : (jh + 1) * 512],
                                wk_sb[:, dt, ht * 128 : (ht + 1) * 128],
                                kT_sb[:, dt, jh * 512 : (jh + 1) * 512],
                                start=(dt == 0),
                                stop=(dt == 1),
                            )
                    nc.vector.tensor_copy(kh_sb[:, ht, :], ps_k)
                ps_q = pp.tile([128, 2 * NQC], F32, tag="ps_q")
                for ht in range(2):
                    for dt in range(2):
                        nc.tensor.matmul(
                            ps_q[:, ht * NQC : (ht + 1) * NQC],
                            wq_sb[:, dt, ht * 128 : (ht + 1) * 128],
                            qT_sb[:, dt, :],
                            start=(dt == 0),
                            stop=(dt == 1),
                        )
                nc.vector.tensor_copy(qh_sb, ps_q)

            # ---- P2: tanh features + transposed score accumulation ----
            with (
                tc.tile_pool(name="feat", bufs=3) as fpool,
                tc.tile_pool(name="sc_psum", bufs=1, space="PSUM") as spool,
            ):
                sT = [spool.tile([128, NQC], F32, tag=f"sT{jt}", name=f"sT{jt}") for jt in range(8)]
                for i in range(NQC):
                    for ht in range(2):
                        feat = fpool.tile([128, NK], BF16, tag=f"feat{ht}", name=f"feat{ht}")
                        nc.scalar.activation(
                            feat,
                            kh_sb[:, ht, :],
                            Tanh,
                            bias=qh_sb[:, ht * NQC + i : ht * NQC + i + 1],
                            scale=1.0,
                        )
                        for jt in range(8):
                            nc.tensor.matmul(
                                sT[jt][:, i : i + 1],
                                feat[:, jt * 128 : (jt + 1) * 128],
                                vw_sb[:, ht : ht + 1],
                                start=(ht == 0),
                                stop=(ht == 1),
                            )

                # ---- P3: exp straight out of PSUM into the transposed layout --
                for jt in range(8):
                    nc.scalar.activation(wT_sb[:, jt, :], sT[jt], Exp)

            # ---- P4/P5: weighted sum of (masked) values + normalize ----
            with tc.tile_pool(name="out_psum", bufs=1, space="PSUM") as opool:
                po = opool.tile([NQC, VA_W], F32)
                for jt in range(8):
                    nc.tensor.matmul(
                        po,
                        wT_sb[:, jt, :],
                        va_sb[:, jt, :],
                        start=(jt == 0),
                        stop=(jt == 7),
                    )
                nc.vector.reciprocal(rsum, po[:, 256:257])
                nc.vector.tensor_scalar_mul(out_sb, po[:, 0:DV], rsum)
                nc.sync.dma_start(out=out_d[:, :], in_=out_sb)

    nc.compile()
    return nc


def _get_nc():
    global _CACHED_NC
    if _CACHED_NC is None:
        _CACHED_NC = build_kernel()
    return _CACHED_NC


def make_in_maps(queries, keys, values, valid_lens, W_q, W_k, v_w):
    wq = np.ascontiguousarray(W_q, np.float32).reshape(2, 128, H).astype(NP_BF16)
    wk = np.ascontiguousarray(W_k, np.float32).reshape(2, 128, H).astype(NP_BF16)
    vw = np.ascontiguousarray(
        np.asarray(v_w, np.float32).reshape(2, 128).T
    ).astype(NP_BF16)
    in_maps = []
    for c in range(N_CORES):
        b, qhalf = divmod(c, 2)
        qs = np.asarray(queries[b, qhalf * NQC : (qhalf + 1) * NQC, :], np.float32)
        qT = np.ascontiguousarray(qs.T).reshape(2, 128, NQC).astype(NP_BF16)
        kT = (
            np.ascontiguousarray(np.asarray(keys[b], np.float32).T)
            .reshape(2, 128, NK)
            .astype(NP_BF16)
        )
        vl = int(valid_lens[b])
        va = np.zeros((NK, VA_W), np.float32)
        va[:vl, :DV] = values[b, :vl]
        va[:vl, DV] = 1.0
        va = va.reshape(8, 128, VA_W).astype(NP_BF16)
        in_maps.append({"qT": qT, "kT": kT, "wq": wq, "wk": wk, "vw": vw, "va": va})
    return in_maps


def run(inputs, trace=False, **kwargs):
    nc = _get_nc()
    in_maps = make_in_maps(**inputs)
    res = run_bass_kernel_spmd(
        nc, in_maps, core_ids=list(range(N_CORES)), trace=trace, **kwargs
    )
    out = np.empty((B, NQ, DV), np.float32)
    for c in range(N_CORES):
        b, qhalf = divmod(c, 2)
        out[b, qhalf * NQC : (qhalf + 1) * NQC, :] = res.results[c]["out"]
    return out, res


def kernel(queries, keys, values, valid_lens, W_q, W_k, v_w):
    out, _ = run(
        dict(
            queries=queries,
            keys=keys,
            values=values,
            valid_lens=valid_lens,
            W_q=W_q,
            W_k=W_k,
            v_w=v_w,
        )
    )
    return out


# revision 3
# speedup vs baseline: 1.3145x; 1.3145x over previous
"""AdditiveAttention kernel for one TRN2 chip (8 NeuronCores).

Reference computation (per batch b):
    q = queries @ W_q                         # (NQ, H)
    k = keys @ W_k                            # (NK, H)
    scores[i,j] = sum_h v_w[h] * tanh(q[i,h] + k[j,h])
    out = masked_softmax(scores, valid_len) @ values

Sharding: data-parallel over (batch, query-half): core c handles batch c//2,
query rows (c%2)*64 .. +64.  All compute is core-local (no collectives);
the host does layout prep (transposes / masking / padding) and reassembly.

Device dataflow per core (64 queries x 1024 keys x H=256):
  P1  kh[h,j] = W_k.T @ keys.T, qh[h,i] = W_q.T @ queries.T     (PE, bf16)
  P2  for each query i: feat = tanh(kh + qh[:,i]) via ScalarE bias-add;
      scoresT[j,i] += feat[h,jblk].T @ v_w  (PE, feat as stationary operand)
      -> scores accumulate TRANSPOSED in PSUM, [j,i] layout
  P3  wT = exp(scoresT)  (no max subtraction needed: |score| <= sum|v_w| ~ 13)
  P4  out_aug[i,:] = sum_j wT[j,i] * va[j,:]  where va = [masked values | mask]
      -> column 256 of out_aug is the softmax denominator
  P5  out = out_aug[:, :256] * (1 / out_aug[:, 256])
"""

import numpy as np
import ml_dtypes

import concourse.bass as bass
import concourse.tile as tile
from concourse import bacc, mybir
from concourse.bass_utils import run_bass_kernel_spmd

BF16 = mybir.dt.bfloat16
F32 = mybir.dt.float32
NP_BF16 = ml_dtypes.bfloat16

B, NQ, NK, DQ, DK, H, DV = 4, 128, 1024, 256, 256, 256, 256
NQC = NQ // 2  # queries per core
VA_W = 258  # values (256) + mask column (1) + pad (1)
N_CORES = 8

_CACHED_NC = None


def build_kernel():
    """Build + compile the per-core Bass graph (SPMD across 8 cores)."""
    nc = bacc.Bacc("TRN2", target_bir_lowering=False, debug=False, num_devices=N_CORES)

    qT_d = nc.declare_dram_parameter("qT", [2, 128, NQC], BF16, isOutput=False)
    kT_d = nc.declare_dram_parameter("kT", [2, 128, NK], BF16, isOutput=False)
    wq_d = nc.declare_dram_parameter("wq", [2, 128, H], BF16, isOutput=False)
    wk_d = nc.declare_dram_parameter("wk", [2, 128, H], BF16, isOutput=False)
    vw_d = nc.declare_dram_parameter("vw", [128, 2], BF16, isOutput=False)
    va_d = nc.declare_dram_parameter("va", [8, 128, VA_W], BF16, isOutput=False)
    out_d = nc.declare_dram_parameter("out", [NQC, DV], F32, isOutput=True)

    Tanh = mybir.ActivationFunctionType.Tanh
    Exp = mybir.ActivationFunctionType.Exp

    with tile.TileContext(nc) as tc:
        with tc.tile_pool(name="const", bufs=1) as cpool:
            wk_sb = cpool.tile([128, 2, H], BF16)
            wq_sb = cpool.tile([128, 2, H], BF16)
            kT_sb = cpool.tile([128, 2, NK], BF16)
            qT_sb = cpool.tile([128, 2, NQC], BF16)
            vw_sb = cpool.tile([128, 2], BF16)
            va_sb = cpool.tile([128, 8, VA_W], BF16)
            for t in range(2):
                nc.sync.dma_start(out=wk_sb[:, t, :], in_=wk_d[t])
                nc.sync.dma_start(out=wq_sb[:, t, :], in_=wq_d[t])
                nc.sync.dma_start(out=kT_sb[:, t, :], in_=kT_d[t])
                nc.sync.dma_start(out=qT_sb[:, t, :], in_=qT_d[t])
            nc.sync.dma_start(out=vw_sb, in_=vw_d[:, :])
            for t in range(8):
                nc.sync.dma_start(out=va_sb[:, t, :], in_=va_d[t])

            kh_sb = cpool.tile([128, 2, NK], BF16)
            qh_sb = cpool.tile([128, 2 * NQC], F32)
            zero_bias = cpool.tile([128, 1], F32)
            nc.vector.memset(zero_bias, 0.0)
            wT_sb = cpool.tile([128, 8, NQC], BF16)
            out_sb = cpool.tile([NQC, DV], F32)
            rsum = cpool.tile([NQC, 1], F32)

            # ---- P1: projections: kh[h, j], qh[h, i] (h on partitions) ----
            with tc.tile_pool(name="proj_psum", bufs=2, space="PSUM") as pp:
                for ht in range(2):
                    ps_k = pp.tile([128, NK], F32, tag="ps_k")
                    for jh in range(2):
                        for dt in range(2):
                            nc.tensor.matmul(
                                ps_k[:, jh * 512 : (jh + 1) * 512],
                                wk_sb[:, dt, ht * 128 : (ht + 1) * 128],
                                kT_sb[:, dt, jh * 512 : (jh + 1) * 512],
                                start=(dt == 0),
                                stop=(dt == 1),
                            )
                    nc.vector.tensor_copy(kh_sb[:, ht, :], ps_k)
                ps_q = pp.tile([128, 2 * NQC], F32, tag="ps_q")
                for ht in range(2):
                    for dt in range(2):
                        nc.tensor.matmul(
                            ps_q[:, ht * NQC : (ht + 1) * NQC],
                            wq_sb[:, dt, ht * 128 : (ht + 1) * 128],
                            qT_sb[:, dt, :],
                            start=(dt == 0),
                            stop=(dt == 1),
                        )
                nc.vector.tensor_copy(qh_sb, ps_q)

            # ---- P2: q-add on VectorE (bf16 4x), giant tanh on ScalarE, ----
            # ----     transposed score accumulation on TensorE          ----
            IB = 4  # queries per tanh block
            with (
                tc.tile_pool(name="feat", bufs=2) as fpool,
                tc.tile_pool(name="sc_psum", bufs=1, space="PSUM") as spool,
            ):
                sT = [spool.tile([128, NQC], F32, tag=f"sT{jt}", name=f"sT{jt}") for jt in range(8)]
                for g in range(NQC // IB):
                    sums = fpool.tile([128, IB * 2 * NK], BF16, tag="sums", name="sums")
                    for ib in range(IB):
                        i = g * IB + ib
                        for ht in range(2):
                            nc.vector.tensor_scalar_add(
                                sums[:, (ib * 2 + ht) * NK : (ib * 2 + ht + 1) * NK],
                                kh_sb[:, ht, :],
                                qh_sb[:, ht * NQC + i : ht * NQC + i + 1],
                            )
                    feat = fpool.tile([128, IB * 2 * NK], BF16, tag="feat", name="feat")
                    nc.scalar.activation(feat, sums, Tanh, bias=zero_bias, scale=1.0)
                    for ib in range(IB):
                        i = g * IB + ib
                        for ht in range(2):
                            off = (ib * 2 + ht) * NK
                            for jt in range(8):
                                nc.tensor.matmul(
                                    sT[jt][:, i : i + 1],
                                    feat[:, off + jt * 128 : off + (jt + 1) * 128],
                                    vw_sb[:, ht : ht + 1],
                                    start=(ht == 0),
                                    stop=(ht == 1),
                                )

                # ---- P3: exp straight out of PSUM into the transposed layout --
                for jt in range(8):
                    nc.scalar.activation(wT_sb[:, jt, :], sT[jt], Exp, bias=zero_bias, scale=1.0)

            # ---- P4/P5: weighted sum of (masked) values + normalize ----
            with tc.tile_pool(name="out_psum", bufs=1, space="PSUM") as opool:
                po = opool.tile([NQC, VA_W], F32)
                for jt in range(8):
                    nc.tensor.matmul(
                        po,
                        wT_sb[:, jt, :],
                        va_sb[:, jt, :],
                        start=(jt == 0),
                        stop=(jt == 7),
                    )
                nc.vector.reciprocal(rsum, po[:, 256:257])
                nc.vector.tensor_scalar_mul(out_sb, po[:, 0:DV], rsum)
                nc.sync.dma_start(out=out_d[:, :], in_=out_sb)

    nc.compile()
    return nc


def _get_nc():
    global _CACHED_NC
    if _CACHED_NC is None:
        _CACHED_NC = build_kernel()
    return _CACHED_NC


def make_in_maps(queries, keys, values, valid_lens, W_q, W_k, v_w):
    wq = np.ascontiguousarray(W_q, np.float32).reshape(2, 128, H).astype(NP_BF16)
    wk = np.ascontiguousarray(W_k, np.float32).reshape(2, 128, H).astype(NP_BF16)
    vw = np.ascontiguousarray(
        np.asarray(v_w, np.float32).reshape(2, 128).T
    ).astype(NP_BF16)
    in_maps = []
    for c in range(N_CORES):
        b, qhalf = divmod(c, 2)
        qs = np.asarray(queries[b, qhalf * NQC : (qhalf + 1) * NQC, :], np.float32)
        qT = np.ascontiguousarray(qs.T).reshape(2, 128, NQC).astype(NP_BF16)
        kT = (
            np.ascontiguousarray(np.asarray(keys[b], np.float32).T)
            .reshape(2, 128, NK)
            .astype(NP_BF16)
        )
        vl = int(valid_lens[b])
        va = np.zeros((NK, VA_W), np.float32)
        va[:vl, :DV] = values[b, :vl]
        va[:vl, DV] = 1.0
        va = va.reshape(8, 128, VA_W).astype(NP_BF16)
        in_maps.append({"qT": qT, "kT": kT, "wq": wq, "wk": wk, "vw": vw, "va": va})
    return in_maps


def run(inputs, trace=False, **kwargs):
    nc = _get_nc()
    in_maps = make_in_maps(**inputs)
    res = run_bass_kernel_spmd(
        nc, in_maps, core_ids=list(range(N_CORES)), trace=trace, **kwargs
    )
    out = np.empty((B, NQ, DV), np.float32)
    for c in range(N_CORES):
        b, qhalf = divmod(c, 2)
        out[b, qhalf * NQC : (qhalf + 1) * NQC, :] = res.results[c]["out"]
    return out, res


def kernel(queries, keys, values, valid_lens, W_q, W_k, v_w):
    out, _ = run(
        dict(
            queries=queries,
            keys=keys,
            values=values,
            valid_lens=valid_lens,
            W_q=W_q,
            W_k=W_k,
            v_w=v_w,
        )
    )
    return out
